# Initial kernel scaffold
#
"""BERT self-attention block (QKV -> attention -> dense -> residual+LN) on 8 trn2 NeuronCores.

Sharding: data-parallel over batch (2) x tensor-parallel over heads (4 heads/core).
Per-core dense partials are summed with a chunked bf16 ReduceScatter over each
batch group ([[0,1,2,3],[4,5,6,7]]); each core finishes residual+LayerNorm on its
own token shard and the host reassembles the full [2, 2048, 1024] output.
"""

import sys

for _p in ("/opt/trn_rl_repo",):
    if _p not in sys.path:
        sys.path.insert(0, _p)

import numpy as np
import ml_dtypes

import concourse.bass as bass
import concourse.mybir as mybir
import concourse.tile as tile
from concourse import bacc
from concourse.bass_utils import run_bass_kernel_spmd

BF16 = ml_dtypes.bfloat16

HIDDEN = 1024
HEADS = 16
HD = 64  # head dim
B = 2
S = 2048
LN_EPS = 1e-5

N_CORES = 8
TP = 4  # tensor-parallel ranks per batch group
LHEADS = HEADS // TP  # 4 local heads
PAIRS = LHEADS // 2  # 2 head pairs
SHARD = S // TP  # 512 tokens of final output per core
NCD = HIDDEN // 128  # 8 contraction chunks
NTOK = S // 128  # 16 token chunks
NQT = 4  # attention q-tiles (512 q each)
QT = S // NQT  # 512
REPLICA_GROUPS = [[0, 1, 2, 3], [4, 5, 6, 7]]
# ReduceScatter chunk boundaries in 128-token units
RS_CHUNKS = [(0, 4), (4, 8), (8, 12), (12, 16)]
NCHUNK = len(RS_CHUNKS)
# per-rank rows per chunk (chunk token count / 4 ranks)
RS_SZ = [(hi - lo) * 32 for lo, hi in RS_CHUNKS]
# padded layout: chunk g's rows live at [g*128, g*128+sz) in hs_res / out
PAD_ROWS = NCHUNK * 128

dt = mybir.dt
Alu = mybir.AluOpType
Act = mybir.ActivationFunctionType


def _build_program(debug_dumps=False):
    nc = bacc.Bacc(
        "TRN2", target_bir_lowering=False, debug=False, num_devices=N_CORES
    )

    # Route Exp and Ln to the one table set that holds both, so the kernel
    # never reloads ACT tables (set ids are positional; only values change).
    from concourse import hw_specs

    for name, funcs in hw_specs.get_activation_tables(nc.m.arch).items():
        if name != "natural_log_exp_and_others":
            funcs.discard(Act.Exp)
            funcs.discard(Act.Ln)

    # ---------------- DRAM I/O ----------------
    hsT = nc.dram_tensor("hsT", [HIDDEN, S], dt.bfloat16, kind="ExternalInput")
    wqk = nc.dram_tensor("wqk", [HIDDEN, 512], dt.bfloat16, kind="ExternalInput")
    wv = nc.dram_tensor("wv", [HIDDEN, 256], dt.bfloat16, kind="ExternalInput")
    wd = nc.dram_tensor("wd", [256, HIDDEN], dt.bfloat16, kind="ExternalInput")
    bqk = nc.dram_tensor("bqk", [512, 1], dt.float32, kind="ExternalInput")
    hs_res = nc.dram_tensor(
        "hs_res", [PAD_ROWS, HIDDEN], dt.float32, kind="ExternalInput"
    )
    out = nc.dram_tensor("out", [PAD_ROWS, HIDDEN], dt.float32, kind="ExternalOutput")

    # internal DRAM for the collective (cannot use I/O tensors)
    cc_in = [
        nc.dram_tensor(f"cc_in{g}", [(hi - lo) * 128, HIDDEN], dt.bfloat16)
        for g, (lo, hi) in enumerate(RS_CHUNKS)
    ]
    cc_out = [
        nc.dram_tensor(f"cc_out{g}", [RS_SZ[g], HIDDEN], dt.bfloat16)
        for g in range(NCHUNK)
    ]
    dumps = {}
    if debug_dumps:
        dumps["qkT"] = nc.dram_tensor(
            "d_qkT", [4, 128, S], dt.bfloat16, kind="ExternalOutput"
        )
        dumps["v"] = nc.dram_tensor(
            "d_v", [NTOK, 128, 512], dt.bfloat16, kind="ExternalOutput"
        )
        dumps["ctxT"] = nc.dram_tensor(
            "d_ctxT", [PAIRS, 128, S], dt.bfloat16, kind="ExternalOutput"
        )

    with tile.TileContext(nc) as tc:
        with (
            tc.tile_pool(name="persist", bufs=1) as persist,
            tc.tile_pool(name="pT_pool", bufs=3) as pT_pool,
            tc.tile_pool(name="work", bufs=3) as work,
            tc.tile_pool(name="ln", bufs=2) as lnp,
            tc.tile_pool(name="psmm", bufs=2, space="PSUM") as psmm,
            tc.tile_pool(name="psctx", bufs=2, space="PSUM") as psctx,
        ):
            # ---------------- persistent SBUF loads ----------------
            zero_sb = persist.tile([128, 1], dt.float32, name="zero_sb")
            nc.vector.memset(zero_sb, 0.0)
            nc.const_aps.aps[(dt.float32, 0.0)] = zero_sb
            eps_sb = persist.tile([128, 1], dt.float32, name="eps_sb")
            nc.vector.memset(eps_sb, LN_EPS)
            # coalesced input DMAs (the sync queue serializes at ~0.6us per
            # dma_start dispatch, so fewer+bigger transfers start compute
            # sooner); hsT is split in two so the first QK matmuls can begin
            # while the second half is still in flight
            hsT_all = persist.tile([128, NCD, S], dt.bfloat16, name="hsT_all")
            hsT_r = hsT[:, :].rearrange("(c p) t -> p c t", p=128)
            nc.sync.dma_start(out=hsT_all[:, 0:4, :], in_=hsT_r[:, 0:4, :])
            wqk_all = persist.tile([128, NCD, 512], dt.bfloat16, name="wqk_all")
            nc.sync.dma_start(
                out=wqk_all, in_=wqk[:, :].rearrange("(c p) n -> p c n", p=128)
            )
            nc.sync.dma_start(out=hsT_all[:, 4:8, :], in_=hsT_r[:, 4:8, :])
            wv_all = persist.tile([128, NCD, 256], dt.bfloat16, name="wv_all")
            nc.sync.dma_start(
                out=wv_all, in_=wv[:, :].rearrange("(c p) n -> p c n", p=128)
            )
            wd_all = persist.tile([128, 2, HIDDEN], dt.bfloat16, name="wd_all")
            nc.sync.dma_start(
                out=wd_all, in_=wd[:, :].rearrange("(c p) n -> p c n", p=128)
            )
            bqk_all = persist.tile([128, 4], dt.float32, name="bqk_all")
            nc.sync.dma_start(
                out=bqk_all, in_=bqk[:, :].rearrange("(m p) o -> p (m o)", p=128)
            )
            res_all = persist.tile([128, NCHUNK, HIDDEN], dt.float32, name="res_all")
            nc.sync.dma_start(
                out=res_all,
                in_=hs_res[:, :].rearrange("(g p) n -> p g n", p=128),
            )
            hsT_sb = [hsT_all[:, c, :] for c in range(NCD)]
            wqk_sb = [wqk_all[:, c, :] for c in range(NCD)]
            wv_sb = [wv_all[:, c, :] for c in range(NCD)]
            wd_sb = [wd_all[:, c, :] for c in range(2)]
            bqk_sb = [bqk_all[:, m : m + 1] for m in range(4)]

            # qkT m-chunk layout: 0=K pair0, 1=Q pair0, 2=K pair1, 3=Q pair1
            # (partitions 0:64 = even head of the pair, 64:128 = odd head)
            qkT_sb = [
                persist.tile([128, S], dt.bfloat16, name=f"qkT{m}") for m in range(4)
            ]
            # V tiles: per token-chunk [128, 512]: 4 groups of [V_h(64) | ones(64)]
            v_sb = [
                persist.tile([128, 512], dt.bfloat16, name=f"v{t}")
                for t in range(NTOK)
            ]
            # ctx^T (normalized, bf16): chunk p holds heads 2p (parts 0:64), 2p+1
            ctxT_sb = [
                persist.tile([128, S], dt.bfloat16, name=f"ctxT{p}")
                for p in range(PAIRS)
            ]

            # ---------------- projection emitters ----------------
            # qkT[m][:, n] = sum_cd wqk[cd, m*128:...]^T @ hsT[cd, n] (+ bias)
            def emit_qk_chunk(m):
                for nh in range(2):  # halves of S
                    ps = psmm.tile([128, 1024], dt.float32, name="ps_mm")
                    for c in range(NCD):
                        for j in range(2):
                            nc.tensor.matmul(
                                ps[:, j * 512 : (j + 1) * 512],
                                lhsT=wqk_sb[c][:, m * 128 : (m + 1) * 128],
                                rhs=hsT_sb[c][
                                    :, nh * 1024 + j * 512 : nh * 1024 + (j + 1) * 512
                                ],
                                start=(c == 0),
                                stop=(c == NCD - 1),
                            )
                    nc.vector.tensor_scalar_add(
                        out=qkT_sb[m][:, nh * 1024 : (nh + 1) * 1024],
                        in0=ps,
                        scalar1=bqk_sb[m],
                    )

            # V[tc][:, l*128 : l*128+64] = hs[tok_chunk] @ wv[:, l*64:...]
            # cols l*128+64 : (l+1)*128 are constant 1.0 (denominator trick)
            def emit_v_chunk(t):
                ps = psmm.tile([128, 1024], dt.float32, name="ps_mm")
                for c in range(NCD):
                    nc.tensor.matmul(
                        ps[:, 0:256],
                        lhsT=hsT_sb[c][:, t * 128 : (t + 1) * 128],
                        rhs=wv_sb[c],
                        start=(c == 0),
                        stop=(c == NCD - 1),
                    )
                vt = v_sb[t].rearrange("p (g c) -> p g c", c=128)
                nc.vector.tensor_copy(
                    out=vt[:, :, 0:64],
                    in_=ps[:, 0:256].rearrange("p (g c) -> p g c", c=64),
                )
                nc.vector.memset(vt[:, :, 64:128], 1.0)

            # all qk chunks plus the first few V chunks up front (q-tile-major
            # attention needs both pairs' operands from the first q-tile on);
            # the remaining V chunks are woven into the first attention tile
            # to fill PE slack while ACT (exp) is the bottleneck
            for m in range(4):
                emit_qk_chunk(m)
            for t in range(4):
                emit_v_chunk(t)

            # ---------------- phase 2: attention + dense + RS ----------------
            # q-tile-major so each RS chunk launches as early as possible --
            # the serialized CC-core queue is the kernel's tail constraint
            cc_insts = []
            dense_state = {"last_evac": None}

            def emit_dense_ti(ti_g):
                tok = ti_g * 128
                ps_d = psmm.tile([128, 1024], dt.float32, name="ps_mm")
                for cc in range(2):
                    for j in range(2):
                        nc.tensor.matmul(
                            ps_d[:, j * 512 : (j + 1) * 512],
                            lhsT=ctxT_sb[cc][:, tok : tok + 128],
                            rhs=wd_sb[cc][:, j * 512 : (j + 1) * 512],
                            start=(cc == 0),
                            stop=(cc == 1),
                        )
                dsb = work.tile([128, 1024], dt.bfloat16, name="dsb")
                dense_state["last_evac"] = nc.vector.tensor_copy(
                    out=dsb, in_=ps_d
                )
                g = next(
                    i for i, (lo, hi) in enumerate(RS_CHUNKS) if lo <= ti_g < hi
                )
                lo = RS_CHUNKS[g][0]
                nc.sync.dma_start(
                    out=cc_in[g][(ti_g - lo) * 128 : (ti_g - lo + 1) * 128, :],
                    in_=dsb,
                )
                if ti_g == RS_CHUNKS[g][1] - 1:
                    cc_insts.append(
                        nc.gpsimd.collective_compute(
                            "ReduceScatter",
                            Alu.add,
                            replica_groups=REPLICA_GROUPS,
                            ins=[cc_in[g][:, :].opt()],
                            outs=[cc_out[g][:, :].opt()],
                        )
                    )

            for qt in range(NQT):
                for p in range(PAIRS):
                    km = 2 * p  # K m-chunk
                    qm = 2 * p + 1  # Q m-chunk
                    ctx_ps = [
                        psctx.tile([128, 512], dt.float32, name=f"ps_ctx{l}")
                        for l in range(2)
                    ]

                    def emit_scores(kc, km=km, qm=qm, qt=qt):
                        ps_s = psmm.tile([128, 1024], dt.float32, name="ps_mm")
                        # scores^T for both heads of the pair (concurrent row
                        # groups: even head rows 0:64, odd head rows 64:128)
                        for l in range(2):
                            nc.tensor.matmul(
                                ps_s[:, l * 512 : (l + 1) * 512],
                                lhsT=qkT_sb[km][
                                    l * 64 : (l + 1) * 64, kc * 128 : (kc + 1) * 128
                                ],
                                rhs=qkT_sb[qm][
                                    l * 64 : (l + 1) * 64, qt * 512 : (qt + 1) * 512
                                ],
                                start=True,
                                stop=True,
                                tile_position=(l * 64, 0),
                            )
                        return ps_s

                    # software pipeline: scores run one k-chunk ahead so the
                    # PE never sits in-order behind ctx(k)'s wait on exp(k)
                    ps_s = emit_scores(0)
                    for kc in range(NTOK):
                        ps_s_next = emit_scores(kc + 1) if kc + 1 < NTOK else None
                        pT = pT_pool.tile([128, 1024], dt.bfloat16, name="pT")
                        nc.scalar.activation(
                            out=pT, in_=ps_s, func=Act.Exp, scale=0.125
                        )
                        ps_s = ps_s_next
                        # ctx^T (+ denominator rows 64:128) accumulate over kc
                        for l in range(2):
                            h = 2 * p + l
                            nc.tensor.matmul(
                                ctx_ps[l],
                                lhsT=v_sb[kc][:, h * 128 : (h + 1) * 128],
                                rhs=pT[:, l * 512 : (l + 1) * 512],
                                start=(kc == 0),
                                stop=(kc == NTOK - 1),
                            )
                        # first q-tile: produce the remaining V chunks just
                        # ahead of their use (ctx(kc) needs v_sb[kc]); later
                        # q-tiles: weave the previous q-tile's dense matmuls
                        # into the PE slack so ACT (exp) never stalls on the
                        # in-order PE queue behind dense work
                        if p == 0 and qt == 0 and kc + 4 < NTOK:
                            emit_v_chunk(kc + 4)
                        # kc>=4 so the previous tile's ctxT normalize (DVE) has
                        # drained before the dense matmuls reach the PE queue
                        if p == 0 and qt >= 1 and kc >= 4 and (kc - 4) % 3 == 0 and (kc - 4) // 3 < 4:
                            emit_dense_ti((qt - 1) * 4 + (kc - 4) // 3)
                    # normalize: ctx[0:64] / den[64:128] -> ctxT (bf16)
                    for l in range(2):
                        den_sb = work.tile([64, 512], dt.float32, name="den_sb")
                        nc.vector.tensor_copy(
                            out=den_sb, in_=ctx_ps[l][64:128, :]
                        )
                        rec = work.tile([64, 512], dt.float32, name="rec")
                        nc.vector.reciprocal_approx_fast(out=rec, in_=den_sb)
                        nc.vector.tensor_tensor(
                            out=ctxT_sb[p][
                                l * 64 : (l + 1) * 64, qt * 512 : (qt + 1) * 512
                            ],
                            in0=ctx_ps[l][0:64, :],
                            in1=rec,
                            op=Alu.mult,
                        )
            # last q-tile's dense has no following attention to hide in
            for ti in range(4):
                emit_dense_ti(12 + ti)
            last_evac = dense_state["last_evac"]

            if debug_dumps:
                for m in range(4):
                    nc.sync.dma_start(out=dumps["qkT"][m, :, :], in_=qkT_sb[m])
                for t in range(NTOK):
                    nc.sync.dma_start(out=dumps["v"][t, :, :], in_=v_sb[t])
                for p in range(PAIRS):
                    nc.sync.dma_start(out=dumps["ctxT"][p, :, :], in_=ctxT_sb[p])
                for q in range(NQT):
                    nc.sync.dma_start(
                        out=dumps["ccin"][q, :, :], in_=cc_in[q][:, :]
                    )
                    nc.sync.dma_start(
                        out=dumps["ccout"][q, :, :], in_=cc_out[q][:, :]
                    )

            # ---------------- phase 3: residual + LayerNorm ----------------
            # Pin every LN chunk after the last dense evacuation so the
            # in-order engine queues never block on an RS mid-attention;
            # LN for the first 3 chunks then fills the final RS wait.
            from concourse.bass import _add_dep_helper

            for g in range(NCHUNK):
                sz = RS_SZ[g]
                xb = lnp.tile([128, HIDDEN], dt.bfloat16, name="xb")
                xb_dma = nc.sync.dma_start(out=xb[:sz, :], in_=cc_out[g][:, :])
                _add_dep_helper(
                    xb_dma.ins,
                    last_evac.ins,
                    sync=True,
                    reason="LN after attention/dense (keep queues unblocked)",
                )
                x = lnp.tile([128, HIDDEN], dt.float32, name="x")
                nc.vector.tensor_tensor(
                    out=x[:sz, :],
                    in0=xb[:sz, :],
                    in1=res_all[:sz, g, :],
                    op=Alu.add,
                )
                stats = lnp.tile([128, 2, 6], dt.float32, name="stats")
                xv = x.rearrange("p (s f) -> p s f", f=512)
                for i in range(2):
                    nc.vector.bn_stats(out=stats[:sz, i, :], in_=xv[:sz, i, :])
                mv = lnp.tile([128, 2], dt.float32, name="mv")
                nc.vector.bn_aggr(out=mv[:sz, :], in_=stats[:sz, :, :])
                # rstd = exp(-0.5 * ln(var + eps)) -- stays in the exp/ln table set
                lnv = lnp.tile([128, 1], dt.float32, name="lnv")
                nc.scalar.activation(
                    out=lnv[:sz, :], in_=mv[:sz, 1:2], func=Act.Ln, bias=eps_sb[:sz, :]
                )
                rstd = lnp.tile([128, 1], dt.float32, name="rstd")
                nc.scalar.activation(
                    out=rstd[:sz, :], in_=lnv[:sz, :], func=Act.Exp, scale=-0.5
                )
                y = lnp.tile([128, HIDDEN], dt.float32, name="y")
                nc.vector.tensor_scalar(
                    out=y[:sz, :],
                    in0=x[:sz, :],
                    scalar1=mv[:sz, 0:1],
                    scalar2=rstd[:sz, :],
                    op0=Alu.subtract,
                    op1=Alu.mult,
                )
                nc.sync.dma_start(
                    out=out[g * 128 : g * 128 + sz, :], in_=y[:sz, :]
                )

    nc.compile()
    return nc


_PROGRAM = None


def _get_program():
    global _PROGRAM
    if _PROGRAM is None:
        _PROGRAM = _build_program()
    return _PROGRAM


def _prep_core_inputs(hidden_states, w_qkv, b_qkv, w_dense, b_dense):
    """Build the 8 per-core input maps (numpy, host-side sharding)."""
    hs = np.asarray(hidden_states, dtype=np.float32)
    w_qkv = np.asarray(w_qkv, dtype=np.float32)
    b_qkv = np.asarray(b_qkv, dtype=np.float32)
    w_dense = np.asarray(w_dense, dtype=np.float32)
    b_dense = np.asarray(b_dense, dtype=np.float32)

    # v-channel bias folded into a host-side output bias:
    # b_out = b_dense + b_v_full @ w_dense   (b_v in ctx channel order)
    bv_full = np.empty((HIDDEN,), dtype=np.float64)
    for g in range(HEADS):
        bv_full[g * HD : (g + 1) * HD] = b_qkv[g * 192 + 128 : g * 192 + 192]
    # w_dense rows are already in (head, d) = g*64+d order, matching bv_full
    b_out = (
        b_dense.astype(np.float64)
        + bv_full @ w_dense.astype(np.float64)
    ).astype(np.float32)

    in_maps = []
    for r in range(N_CORES):
        b = r // TP
        tp = r % TP
        gheads = [4 * tp + l for l in range(LHEADS)]

        hsT_bf = np.ascontiguousarray(hs[b].T).astype(BF16)  # [1024, 2048]

        # wqk column order: per pair: K(even) K(odd) Q(even) Q(odd), 64 each
        wqk_cols = np.empty((HIDDEN, 512), dtype=np.float32)
        bqk_vec = np.empty((512,), dtype=np.float32)
        for p in range(PAIRS):
            for l in range(2):
                g = gheads[2 * p + l]
                kcol = slice(g * 192 + 64, g * 192 + 128)
                qcol = slice(g * 192, g * 192 + 64)
                base = p * 256
                wqk_cols[:, base + l * 64 : base + (l + 1) * 64] = w_qkv[:, kcol]
                wqk_cols[:, base + 128 + l * 64 : base + 128 + (l + 1) * 64] = w_qkv[
                    :, qcol
                ]
                bqk_vec[base + l * 64 : base + (l + 1) * 64] = b_qkv[kcol]
                bqk_vec[base + 128 + l * 64 : base + 128 + (l + 1) * 64] = b_qkv[qcol]

        wv_cols = np.empty((HIDDEN, 256), dtype=np.float32)
        for l, g in enumerate(gheads):
            wv_cols[:, l * 64 : (l + 1) * 64] = w_qkv[
                :, g * 192 + 128 : g * 192 + 192
            ]

        wd_rows = np.empty((256, HIDDEN), dtype=np.float32)
        for l, g in enumerate(gheads):
            wd_rows[l * 64 : (l + 1) * 64, :] = w_dense[g * 64 : (g + 1) * 64, :]

        # residual shard (+ folded output bias); padded layout: chunk g's
        # sz rows live at [g*128, g*128+sz), covering global tokens
        # lo*128 + tp*sz + [0, sz)
        res = np.zeros((PAD_ROWS, HIDDEN), dtype=np.float32)
        for g, (lo, hi) in enumerate(RS_CHUNKS):
            sz = RS_SZ[g]
            t0 = lo * 128 + tp * sz
            res[g * 128 : g * 128 + sz, :] = hs[b, t0 : t0 + sz, :] + b_out

        in_maps.append(
            {
                "hsT": hsT_bf,
                "wqk": wqk_cols.astype(BF16),
                "wv": wv_cols.astype(BF16),
                "wd": wd_rows.astype(BF16),
                "bqk": bqk_vec.reshape(512, 1),
                "hs_res": res,
            }
        )
    return in_maps


def kernel(hidden_states, w_qkv, b_qkv, w_dense, b_dense, ln_gamma, ln_beta,
           _return_perf=False, **run_kwargs):
    ln_gamma = np.asarray(ln_gamma, dtype=np.float32)
    ln_beta = np.asarray(ln_beta, dtype=np.float32)
    gamma_one = np.allclose(ln_gamma, 1.0)
    beta_zero = np.allclose(ln_beta, 0.0)

    nc = _get_program()
    in_maps = _prep_core_inputs(hidden_states, w_qkv, b_qkv, w_dense, b_dense)
    res = run_bass_kernel_spmd(
        nc, in_maps, core_ids=list(range(N_CORES)), **run_kwargs
    )

    full = np.empty((B, S, HIDDEN), dtype=np.float32)
    for r in range(N_CORES):
        b = r // TP
        tp = r % TP
        o = res.results[r]["out"]
        for g, (lo, hi) in enumerate(RS_CHUNKS):
            sz = RS_SZ[g]
            t0 = lo * 128 + tp * sz
            full[b, t0 : t0 + sz, :] = o[g * 128 : g * 128 + sz, :]

    if not (gamma_one and beta_zero):
        # spec fills gamma=ones, beta=zeros; fall back on host if they differ
        full = full * ln_gamma[None, None, :] + ln_beta[None, None, :]

    if _return_perf:
        return full, res
    return full



# revision 29
# speedup vs baseline: 1.3692x; 1.3692x over previous
"""BERT self-attention block (QKV -> attention -> dense -> residual+LN) on 8 trn2 NeuronCores.

Sharding: data-parallel over batch (2) x tensor-parallel over heads (4 heads/core).
Per-core dense partials are summed with a chunked fp8 ReduceScatter over each
batch group ([[0,1,2,3],[4,5,6,7]]); each core finishes residual+LayerNorm on its
own token shard and the host reassembles the full [2, 2048, 1024] output.

All matmuls except the 64-deep QK^T scores run as fp8e4 DoubleRow (two
contraction elements per PE cell per cycle -> 2x throughput); scores stay bf16
since a 64-deep contraction cannot use the extra rows.  Weights are pre-scaled
x32 on the host so fp8 operands sit in e4m3's normal range, and the scale is
divided back out by the exp scale (scores), the denominator ratio (V path),
and the dense evacuation (x1/32) + scale-invariant LayerNorm (output path).
"""

import sys

for _p in ("/opt/trn_rl_repo",):
    if _p not in sys.path:
        sys.path.insert(0, _p)

import numpy as np
import ml_dtypes

import concourse.bass as bass
import concourse.mybir as mybir
import concourse.tile as tile
from concourse import bacc
from concourse.bass_utils import run_bass_kernel_spmd

BF16 = ml_dtypes.bfloat16
FP8 = ml_dtypes.float8_e4m3  # TRN float8e4: max normal 240, like IEEE e4m3

HIDDEN = 1024
HEADS = 16
HD = 64  # head dim
B = 2
S = 2048
LN_EPS = 1e-5

N_CORES = 8
TP = 4  # tensor-parallel ranks per batch group
LHEADS = HEADS // TP  # 4 local heads
PAIRS = LHEADS // 2  # 2 head pairs
SHARD = S // TP  # 512 tokens of final output per core
NCD = HIDDEN // 128  # 8 contraction chunks
NTOK = S // 128  # 16 key chunks
NT2 = NTOK // 2  # 8 DoubleRow key chunk-pairs
NQT = 4  # attention q-tiles (512 q each)
QT = S // NQT  # 512
REPLICA_GROUPS = [[0, 1, 2, 3], [4, 5, 6, 7]]
# ReduceScatter chunk boundaries in 128-token units.  4-rank collectives run
# the ring algorithm whose ~20us step floor dominates small payloads, so fewer
# bigger chunks beat many small ones; the tail is one 0.5MB fp8 RS.
RS_CHUNKS = [(0, 4), (4, 8), (8, 12), (12, 16)]
NCHUNK = len(RS_CHUNKS)
# per-rank rows per chunk (chunk token count / 4 ranks)
RS_SZ = [(hi - lo) * 32 for lo, hi in RS_CHUNKS]
# padded layout: chunk g's rows live at [g*128, g*128+sz) in hs_res / out
PAD_ROWS = NCHUNK * 128
# fp8 operand/wire scale: weights x32 host-side so fp8 values clear e4m3's
# subnormal cutoff; the dense ReduceScatter payload is 32x the true partial
# and scale-invariant LayerNorm (with eps x32^2) divides it back out
W_SCALE = 32.0
CC_SCALE = 32.0
# scores psum holds (32q)dot(32k); exp folds the /8 softmax scale and /1024
SCORE_SCALE = 0.125 / (W_SCALE * W_SCALE)
# Schraudolph exp-via-int-bits constants (y = bitcast_f32(round(x*A + B)) ~
# e^(x*SCORE_SCALE), rel err ~3%): offloads some exps from the saturated ACT
# engine to DVE (int mul-add) + GPSIMD (fp8 convert of the bitcast view)
SCH_A = float((1 << 23) * SCORE_SCALE / np.log(2.0))
SCH_B = float((1 << 23) * (127.0 - 0.0434609))

dt = mybir.dt
Alu = mybir.AluOpType
Act = mybir.ActivationFunctionType
DR = mybir.MatmulPerfMode.DoubleRow


def _build_program(debug_dumps=False):
    nc = bacc.Bacc(
        "TRN2", target_bir_lowering=False, debug=False, num_devices=N_CORES
    )

    # Route Exp and Ln to the one table set that holds both, so the kernel
    # never reloads ACT tables (set ids are positional; only values change).
    from concourse import hw_specs

    for name, funcs in hw_specs.get_activation_tables(nc.m.arch).items():
        if name != "natural_log_exp_and_others":
            funcs.discard(Act.Exp)
            funcs.discard(Act.Ln)

    # ---------------- DRAM I/O ----------------
    hsT = nc.dram_tensor("hsT", [HIDDEN, S], dt.float8e4, kind="ExternalInput")
    wqk = nc.dram_tensor("wqk", [HIDDEN, 512], dt.float8e4, kind="ExternalInput")
    wv = nc.dram_tensor("wv", [HIDDEN, 256], dt.float8e4, kind="ExternalInput")
    # w_dense pre-arranged for DoubleRow: [chan-in-pair 128, pair 2, hid 1024]
    wd = nc.dram_tensor("wd", [128, 2 * HIDDEN], dt.float8e4, kind="ExternalInput")
    bqk = nc.dram_tensor("bqk", [512, 1], dt.float32, kind="ExternalInput")
    hs_res = nc.dram_tensor(
        "hs_res", [PAD_ROWS, HIDDEN], dt.float32, kind="ExternalInput"
    )
    out = nc.dram_tensor("out", [PAD_ROWS, HIDDEN], dt.float32, kind="ExternalOutput")

    # internal DRAM for the collective (cannot use I/O tensors)
    cc_in = [
        nc.dram_tensor(f"cc_in{g}", [(hi - lo) * 128, HIDDEN], dt.float8e4)
        for g, (lo, hi) in enumerate(RS_CHUNKS)
    ]
    cc_out = [
        nc.dram_tensor(f"cc_out{g}", [RS_SZ[g], HIDDEN], dt.float8e4)
        for g in range(NCHUNK)
    ]

    with tile.TileContext(nc) as tc:
        with (
            tc.tile_pool(name="persist", bufs=1) as persist,
            tc.tile_pool(name="pT_pool", bufs=3) as pT_pool,
            tc.tile_pool(name="work", bufs=3) as work,
            tc.tile_pool(name="ln", bufs=2) as lnp,
            tc.tile_pool(name="psmm", bufs=2, space="PSUM") as psmm,
            tc.tile_pool(name="psq", bufs=2, space="PSUM") as psq,
            tc.tile_pool(name="psctx", bufs=1, space="PSUM") as psctx,
        ):
            # ---------------- persistent SBUF loads ----------------
            zero_sb = persist.tile([128, 1], dt.float32, name="zero_sb")
            nc.vector.memset(zero_sb, 0.0)
            nc.const_aps.aps[(dt.float32, 0.0)] = zero_sb
            eps_sb = persist.tile([128, 1], dt.float32, name="eps_sb")
            nc.vector.memset(eps_sb, LN_EPS * CC_SCALE * CC_SCALE)
            inv32_sb = persist.tile([128, 1], dt.float32, name="inv32_sb")
            nc.vector.memset(inv32_sb, 1.0 / 32.0)
            # input DMAs ordered by when compute needs them: the first QK
            # quarter needs all hidden chunks of q-tile 0's tokens plus its
            # wqk column block, so hsT is split by TOKENS and wqk goes first;
            # wd/res aren't needed until the dense/LN phases and go last
            bqk_all = persist.tile([128, 4], dt.float32, name="bqk_all")
            nc.sync.dma_start(
                out=bqk_all, in_=bqk[:, :].rearrange("(m p) o -> p (m o)", p=128)
            )
            wqk_all = persist.tile([128, NCD, 512], dt.float8e4, name="wqk_all")
            wqk_r = wqk[:, :].rearrange("(c p) n -> p c n", p=128)
            nc.sync.dma_start(out=wqk_all[:, :, 0:256], in_=wqk_r[:, :, 0:256])
            hsT_all = persist.tile([128, NCD, S], dt.float8e4, name="hsT_all")
            hsT_r = hsT[:, :].rearrange("(c p) t -> p c t", p=128)
            nc.sync.dma_start(out=hsT_all[:, :, 0:512], in_=hsT_r[:, :, 0:512])
            wv_all = persist.tile([128, NCD, 256], dt.float8e4, name="wv_all")
            nc.sync.dma_start(
                out=wv_all, in_=wv[:, :].rearrange("(c p) n -> p c n", p=128)
            )
            nc.sync.dma_start(out=hsT_all[:, :, 512:1024], in_=hsT_r[:, :, 512:1024])
            nc.sync.dma_start(out=hsT_all[:, :, 1024:2048], in_=hsT_r[:, :, 1024:2048])
            nc.sync.dma_start(out=wqk_all[:, :, 256:512], in_=wqk_r[:, :, 256:512])
            wd2_all = persist.tile([128, 2, HIDDEN], dt.float8e4, name="wd2_all")
            nc.sync.dma_start(
                out=wd2_all, in_=wd[:, :].rearrange("p (i n) -> p i n", i=2)
            )
            res_all = persist.tile([128, NCHUNK, HIDDEN], dt.float32, name="res_all")
            nc.sync.dma_start(
                out=res_all,
                in_=hs_res[:, :].rearrange("(g p) n -> p g n", p=128),
            )
            bqk_sb = [bqk_all[:, m : m + 1] for m in range(4)]
            # DoubleRow views: pair hidden chunk j with chunk j+4 so one pass
            # contracts 256 hidden dims (128 partitions x 2 per cell)
            hsT_dr = hsT_all.rearrange("p (i c) t -> p i c t", i=2)
            wqk_dr = wqk_all.rearrange("p (i c) n -> p i c n", i=2)
            wv_dr = wv_all.rearrange("p (i c) n -> p i c n", i=2)

            # qkT m-chunk layout: 0=K pair0, 1=Q pair0, 2=K pair1, 3=Q pair1
            # (partitions 0:64 = even head of the pair, 64:128 = odd head);
            # values are 32x-scaled (folded into wqk/bqk host-side)
            qkT_sb = [
                persist.tile([128, S], dt.bfloat16, name=f"qkT{m}") for m in range(4)
            ]
            # V tiles per key chunk-PAIR: [key-in-chunk 128, chunk-parity 2,
            # 4 heads x (32V(64) | ones(64))] in fp8 for DoubleRow ctx
            v2_sb = [
                persist.tile([128, 2, 512], dt.float8e4, name=f"v{t}")
                for t in range(NT2)
            ]
            # ctx^T fp8, 32x-scaled: [chan-in-pair 128, pair 2, tok S]
            ctxT2 = persist.tile([128, 2, S], dt.float8e4, name="ctxT2")

            # ---------------- projection emitters ----------------
            def emit_qk_quarter(m, q):
                ps = psq.tile([128, 512], dt.float32, name="ps_qk")
                for j in range(4):
                    nc.tensor.matmul(
                        ps,
                        lhsT=wqk_dr[:, :, j, m * 128 : (m + 1) * 128],
                        rhs=hsT_dr[:, :, j, q * 512 : (q + 1) * 512],
                        start=(j == 0),
                        stop=(j == 3),
                        perf_mode=DR,
                    )
                nc.vector.tensor_scalar_add(
                    out=qkT_sb[m][:, q * 512 : (q + 1) * 512],
                    in0=ps,
                    scalar1=bqk_sb[m],
                )

            def emit_v2_chunk(t2):
                for i in range(2):
                    t = 2 * t2 + i
                    ps = psq.tile([128, 512], dt.float32, name="ps_qk")
                    for j in range(4):
                        nc.tensor.matmul(
                            ps[:, 0:256],
                            lhsT=hsT_dr[:, :, j, t * 128 : (t + 1) * 128],
                            rhs=wv_dr[:, :, j, :],
                            start=(j == 0),
                            stop=(j == 3),
                            perf_mode=DR,
                        )
                    vt = v2_sb[t2][:, i, :].rearrange("p (g c) -> p g c", c=128)
                    nc.vector.tensor_copy(
                        out=vt[:, :, 0:64],
                        in_=ps[:, 0:256].rearrange("p (g c) -> p g c", c=64),
                    )
                vts = v2_sb[t2].rearrange("p i (g c) -> p i g c", c=128)
                nc.vector.memset(vts[:, :, :, 64:128], 1.0)

            # PE warmup: the HAM clock gate keeps an idle PE at half clock
            # and takes ~3.4us of sustained activity to release; burn dummy
            # matmuls during the input-DMA wait so the real stream runs warm
            dummy_sb = persist.tile([128, 512], dt.bfloat16, name="dummy_sb")
            nc.vector.memset(dummy_sb, 0.0)
            for _ in range(12):
                ps_w = psq.tile([128, 512], dt.float32, name="ps_qk")
                nc.tensor.matmul(
                    ps_w[0:1, :], lhsT=zero_sb[:, :].bitcast(dt.bfloat16)[:, 0:1],
                    rhs=dummy_sb, start=True, stop=True,
                )
            # Minimum prefix before attention can start: K pair0 q-tile 0
            # (covers scores kc 0..3) and Q pair0 q-tile 0; everything else
            # is woven into the attention loops' PE slack so exp starts as
            # soon as the first token quarter lands.
            emit_qk_quarter(0, 0)
            emit_qk_quarter(1, 0)

            # ---------------- phase 2: attention + dense + RS ----------------
            cc_insts = []
            dense_state = {"last_evac": None}

            def emit_dense_ti(ti_g):
                tok = ti_g * 128
                dsb = work.tile([128, 1024], dt.float8e4, name="dsb")
                for j in range(2):
                    ps_d = psq.tile([128, 512], dt.float32, name="ps_qk")
                    nc.tensor.matmul(
                        ps_d,
                        lhsT=ctxT2[:, :, tok : tok + 128],
                        rhs=wd2_all[:, :, j * 512 : (j + 1) * 512],
                        start=True,
                        stop=True,
                        perf_mode=DR,
                    )
                    # psum = (32 ctx)(32 wd) = 1024x partial; wire wants 32x
                    dense_state["last_evac"] = nc.vector.tensor_scalar_mul(
                        out=dsb[:, j * 512 : (j + 1) * 512],
                        in0=ps_d,
                        scalar1=1.0 / 32.0,
                    )
                g = next(
                    i for i, (lo, hi) in enumerate(RS_CHUNKS) if lo <= ti_g < hi
                )
                lo = RS_CHUNKS[g][0]
                nc.sync.dma_start(
                    out=cc_in[g][(ti_g - lo) * 128 : (ti_g - lo + 1) * 128, :],
                    in_=dsb,
                )
                if ti_g == RS_CHUNKS[g][1] - 1:
                    cc_insts.append(
                        nc.gpsimd.collective_compute(
                            "ReduceScatter",
                            Alu.add,
                            replica_groups=REPLICA_GROUPS,
                            ins=[cc_in[g][:, :].opt()],
                            outs=[cc_out[g][:, :].opt()],
                        )
                    )

            # filler schedule: (qt, pair, kc) -> callables emitting ~0.9us of
            # PE work each, consumed after that kc-pair's ctx matmuls.
            # V chunk-pairs must precede their use in qt0-pair0's ctx; qk
            # quarters must precede the (qt, pair) that reads them; dense
            # ti's trail their q-tile by one qt.
            fill = {}

            def _add(qt, p, kc, fn):
                fill.setdefault((qt, p, kc), []).append(fn)

            for t2 in range(6):  # V chunk-pairs 2..7 during qt0-pair0
                _add(0, 0, 2 * t2, (lambda t=t2 + 2: emit_v2_chunk(t)))
            for kc, (m, q) in [
                (0, (0, 1)), (1, (2, 1)), (2, (0, 2)), (3, (0, 3)),
                (4, (2, 0)), (6, (3, 0)),
            ]:
                _add(0, 0, kc, (lambda m=m, q=q: emit_qk_quarter(m, q)))
            for kc, (m, q) in [
                (0, (2, 2)), (1, (1, 1)), (2, (3, 1)), (4, (2, 3)),
            ]:
                _add(0, 1, kc, (lambda m=m, q=q: emit_qk_quarter(m, q)))
            _add(1, 0, 5, lambda: emit_qk_quarter(1, 2))
            _add(1, 1, 4, lambda: emit_qk_quarter(3, 2))
            _add(2, 0, 5, lambda: emit_qk_quarter(1, 3))
            _add(2, 1, 4, lambda: emit_qk_quarter(3, 3))
            # dense for q-tile qt woven into qt+1 pair0 (kc>=4 so the
            # previous tile's ctxT normalize on DVE has drained first)
            for qt in range(1, NQT):
                for i in range(4):
                    _add(qt, 0, 4 + 2 * i, (lambda ti=(qt - 1) * 4 + i: emit_dense_ti(ti)))

            # Whole kc-pairs offloaded from the saturated ACT engine to DVE:
            # exp is computed as Schraudolph int-bits on DVE and the ctx
            # matmul reads the int32 buffer through a truncated-bf16 view
            # (the high half of each fp32), so no convert op is needed.
            # t2=3 of every (qt, pair) except qt0-pair0 (its DVE is already
            # loaded with the woven V2 copies).
            OFF_T2 = 3
            offload = set()  # measured: DVE can't absorb the work in-window
            ibs = {}

            def emit_exp(pT_slice, ps, qt, p, kc):
                if (qt, p) in offload and kc // 2 == OFF_T2:
                    ib = work.tile([128, 1024], dt.int32, name="schb")
                    nc.vector.tensor_scalar(
                        out=ib, in0=ps, scalar1=SCH_A, scalar2=SCH_B,
                        op0=Alu.mult, op1=Alu.add,
                    )
                    ibs[(qt, p, kc)] = ib
                else:
                    nc.scalar.activation(
                        out=pT_slice, in_=ps, func=Act.Exp, scale=SCORE_SCALE
                    )

            for qt in range(NQT):
                for p in range(PAIRS):
                    km = 2 * p  # K m-chunk
                    qm = 2 * p + 1  # Q m-chunk
                    ctx_ps = [
                        psctx.tile([128, 512], dt.float32, name=f"ps_ctx{l}")
                        for l in range(2)
                    ]

                    def emit_scores(kc, km=km, qm=qm, qt=qt):
                        ps_s = psmm.tile([128, 1024], dt.float32, name="ps_mm")
                        # scores^T for both heads of the pair (concurrent row
                        # groups: even head rows 0:64, odd head rows 64:128)
                        for l in range(2):
                            nc.tensor.matmul(
                                ps_s[:, l * 512 : (l + 1) * 512],
                                lhsT=qkT_sb[km][
                                    l * 64 : (l + 1) * 64, kc * 128 : (kc + 1) * 128
                                ],
                                rhs=qkT_sb[qm][
                                    l * 64 : (l + 1) * 64, qt * 512 : (qt + 1) * 512
                                ],
                                start=True,
                                stop=True,
                                tile_position=(l * 64, 0),
                            )
                        return ps_s

                    # software pipeline: scores run one k-chunk ahead so the
                    # PE never sits in-order behind ctx's wait on exp
                    ps_a = emit_scores(0)
                    ps_b = emit_scores(1)
                    if qt == 0 and p == 0:
                        emit_v2_chunk(0)
                        emit_v2_chunk(1)
                    for t2 in range(NT2):
                        pT2 = pT_pool.tile([128, 2, 1024], dt.float8e4, name="pT")
                        emit_exp(pT2[:, 0, :], ps_a, qt, p, 2 * t2)
                        ps_a = (
                            emit_scores(2 * t2 + 2) if 2 * t2 + 2 < NTOK else None
                        )
                        emit_exp(pT2[:, 1, :], ps_b, qt, p, 2 * t2 + 1)
                        ps_b = (
                            emit_scores(2 * t2 + 3) if 2 * t2 + 3 < NTOK else None
                        )
                        # ctx^T (+ denominator rows 64:128) over the key
                        # chunk-pair: DoubleRow contracts 256 keys per pass
                        if (qt, p) in offload and t2 == OFF_T2:
                            for i in range(2):
                                ib = ibs.pop((qt, p, 2 * t2 + i))
                                pbf = ib[:, :].bitcast(dt.bfloat16).rearrange(
                                    "q (f two) -> q f two", two=2
                                )[:, :, 1]
                                for l in range(2):
                                    h = 2 * p + l
                                    nc.tensor.matmul(
                                        ctx_ps[l],
                                        lhsT=v2_sb[t2][:, i, h * 128 : (h + 1) * 128],
                                        rhs=pbf[:, l * 512 : (l + 1) * 512],
                                        start=False,
                                        stop=False,
                                    )
                        else:
                            for l in range(2):
                                h = 2 * p + l
                                nc.tensor.matmul(
                                    ctx_ps[l],
                                    lhsT=v2_sb[t2][:, :, h * 128 : (h + 1) * 128],
                                    rhs=pT2[:, :, l * 512 : (l + 1) * 512],
                                    start=(t2 == 0),
                                    stop=(t2 == NT2 - 1),
                                    perf_mode=DR,
                                )
                        for fn in fill.get((qt, p, 2 * t2), ()):
                            fn()
                        for fn in fill.get((qt, p, 2 * t2 + 1), ()):
                            fn()
                    # normalize: 32V num [0:64] / den [64:128] -> ctxT2 (fp8)
                    for l in range(2):
                        den_sb = work.tile([64, 512], dt.float32, name="den_sb")
                        nc.vector.tensor_copy(
                            out=den_sb, in_=ctx_ps[l][64:128, :]
                        )
                        rec = work.tile([64, 512], dt.float32, name="rec")
                        nc.vector.reciprocal_approx_fast(out=rec, in_=den_sb)
                        nc.vector.tensor_tensor(
                            out=ctxT2[
                                l * 64 : (l + 1) * 64, p, qt * 512 : (qt + 1) * 512
                            ],
                            in0=ctx_ps[l][0:64, :],
                            in1=rec,
                            op=Alu.mult,
                        )
            # last q-tile's dense has no following attention to hide in
            for ti in range(4):
                emit_dense_ti(12 + ti)
            last_evac = dense_state["last_evac"]

            # ---------------- phase 3: residual + LayerNorm ----------------
            # Pin every LN chunk after the last dense evacuation so the
            # in-order engine queues never block on an RS mid-attention;
            # LN for the earlier chunks then fills the final RS wait.
            from concourse.bass import _add_dep_helper

            for g in range(NCHUNK):
                sz = RS_SZ[g]
                xb = lnp.tile([128, HIDDEN], dt.float8e4, name="xb")
                xb_dma = nc.sync.dma_start(out=xb[:sz, :], in_=cc_out[g][:, :])
                _add_dep_helper(
                    xb_dma.ins,
                    last_evac.ins,
                    sync=True,
                    reason="LN after attention/dense (keep queues unblocked)",
                )
                x = lnp.tile([128, HIDDEN], dt.float32, name="x")
                nc.vector.tensor_tensor(
                    out=x[:sz, :],
                    in0=xb[:sz, :],
                    in1=res_all[:sz, g, :],
                    op=Alu.add,
                )
                stats = lnp.tile([128, 2, 6], dt.float32, name="stats")
                xv = x.rearrange("p (s f) -> p s f", f=512)
                for i in range(2):
                    nc.vector.bn_stats(out=stats[:sz, i, :], in_=xv[:sz, i, :])
                mv = lnp.tile([128, 2], dt.float32, name="mv")
                nc.vector.bn_aggr(out=mv[:sz, :], in_=stats[:sz, :, :])
                # rstd = exp(-0.5 * ln(var + eps)) -- stays in the exp/ln table set
                lnv = lnp.tile([128, 1], dt.float32, name="lnv")
                nc.scalar.activation(
                    out=lnv[:sz, :], in_=mv[:sz, 1:2], func=Act.Ln, bias=eps_sb[:sz, :]
                )
                rstd = lnp.tile([128, 1], dt.float32, name="rstd")
                nc.scalar.activation(
                    out=rstd[:sz, :], in_=lnv[:sz, :], func=Act.Exp, scale=-0.5
                )
                y = lnp.tile([128, HIDDEN], dt.float32, name="y")
                nc.vector.tensor_scalar(
                    out=y[:sz, :],
                    in0=x[:sz, :],
                    scalar1=mv[:sz, 0:1],
                    scalar2=rstd[:sz, :],
                    op0=Alu.subtract,
                    op1=Alu.mult,
                )
                nc.sync.dma_start(
                    out=out[g * 128 : g * 128 + sz, :], in_=y[:sz, :]
                )

    nc.compile()
    return nc


_PROGRAM = None


def _get_program():
    global _PROGRAM
    if _PROGRAM is None:
        _PROGRAM = _build_program()
    return _PROGRAM


def _prep_core_inputs(hidden_states, w_qkv, b_qkv, w_dense, b_dense):
    """Build the 8 per-core input maps (numpy, host-side sharding)."""
    hs = np.asarray(hidden_states, dtype=np.float32)
    w_qkv = np.asarray(w_qkv, dtype=np.float32)
    b_qkv = np.asarray(b_qkv, dtype=np.float32)
    w_dense = np.asarray(w_dense, dtype=np.float32)
    b_dense = np.asarray(b_dense, dtype=np.float32)

    # v-channel bias folded into a host-side output bias:
    # b_out = b_dense + b_v_full @ w_dense   (b_v in ctx channel order)
    bv_full = np.empty((HIDDEN,), dtype=np.float64)
    for g in range(HEADS):
        bv_full[g * HD : (g + 1) * HD] = b_qkv[g * 192 + 128 : g * 192 + 192]
    # w_dense rows are already in (head, d) = g*64+d order, matching bv_full
    b_out = (
        b_dense.astype(np.float64)
        + bv_full @ w_dense.astype(np.float64)
    ).astype(np.float32)

    in_maps = []
    for r in range(N_CORES):
        b = r // TP
        tp = r % TP
        gheads = [4 * tp + l for l in range(LHEADS)]

        hsT_f8 = np.ascontiguousarray(hs[b].T).astype(FP8)  # [1024, 2048]

        # wqk column order: per pair: K(even) K(odd) Q(even) Q(odd), 64 each
        wqk_cols = np.empty((HIDDEN, 512), dtype=np.float32)
        bqk_vec = np.empty((512,), dtype=np.float32)
        for p in range(PAIRS):
            for l in range(2):
                g = gheads[2 * p + l]
                kcol = slice(g * 192 + 64, g * 192 + 128)
                qcol = slice(g * 192, g * 192 + 64)
                base = p * 256
                wqk_cols[:, base + l * 64 : base + (l + 1) * 64] = w_qkv[:, kcol]
                wqk_cols[:, base + 128 + l * 64 : base + 128 + (l + 1) * 64] = w_qkv[
                    :, qcol
                ]
                bqk_vec[base + l * 64 : base + (l + 1) * 64] = b_qkv[kcol]
                bqk_vec[base + 128 + l * 64 : base + 128 + (l + 1) * 64] = b_qkv[qcol]

        wv_cols = np.empty((HIDDEN, 256), dtype=np.float32)
        for l, g in enumerate(gheads):
            wv_cols[:, l * 64 : (l + 1) * 64] = w_qkv[
                :, g * 192 + 128 : g * 192 + 192
            ]

        # head-ordered dense rows, DoubleRow layout [chan-in-pair, pair, hid]
        wd_rows = np.empty((256, HIDDEN), dtype=np.float32)
        for l, g in enumerate(gheads):
            wd_rows[l * 64 : (l + 1) * 64, :] = w_dense[g * 64 : (g + 1) * 64, :]
        wd2 = np.ascontiguousarray(
            (wd_rows * W_SCALE).reshape(2, 128, HIDDEN).transpose(1, 0, 2)
        ).reshape(128, 2 * HIDDEN)

        # residual shard (+ folded output bias); padded layout: chunk g's
        # sz rows live at [g*128, g*128+sz), covering global tokens
        # lo*128 + tp*sz + [0, sz)
        res = np.zeros((PAD_ROWS, HIDDEN), dtype=np.float32)
        for g, (lo, hi) in enumerate(RS_CHUNKS):
            sz = RS_SZ[g]
            t0 = lo * 128 + tp * sz
            res[g * 128 : g * 128 + sz, :] = (
                hs[b, t0 : t0 + sz, :] + b_out
            ) * CC_SCALE

        in_maps.append(
            {
                "hsT": hsT_f8,
                "wqk": (wqk_cols * W_SCALE).astype(FP8),
                "wv": (wv_cols * W_SCALE).astype(FP8),
                "wd": wd2.astype(FP8),
                "bqk": (bqk_vec * W_SCALE).reshape(512, 1),
                "hs_res": res,
            }
        )
    return in_maps


def kernel(hidden_states, w_qkv, b_qkv, w_dense, b_dense, ln_gamma, ln_beta,
           _return_perf=False, **run_kwargs):
    ln_gamma = np.asarray(ln_gamma, dtype=np.float32)
    ln_beta = np.asarray(ln_beta, dtype=np.float32)
    gamma_one = np.allclose(ln_gamma, 1.0)
    beta_zero = np.allclose(ln_beta, 0.0)

    nc = _get_program()
    in_maps = _prep_core_inputs(hidden_states, w_qkv, b_qkv, w_dense, b_dense)
    res = run_bass_kernel_spmd(
        nc, in_maps, core_ids=list(range(N_CORES)), **run_kwargs
    )

    full = np.empty((B, S, HIDDEN), dtype=np.float32)
    for r in range(N_CORES):
        b = r // TP
        tp = r % TP
        o = res.results[r]["out"]
        for g, (lo, hi) in enumerate(RS_CHUNKS):
            sz = RS_SZ[g]
            t0 = lo * 128 + tp * sz
            full[b, t0 : t0 + sz, :] = o[g * 128 : g * 128 + sz, :]

    if not (gamma_one and beta_zero):
        # spec fills gamma=ones, beta=zeros; fall back on host if they differ
        full = full * ln_gamma[None, None, :] + ln_beta[None, None, :]

    if _return_perf:
        return full, res
    return full


# revision 30
# speedup vs baseline: 1.4051x; 1.0262x over previous
"""BERT self-attention block (QKV -> attention -> dense -> residual+LN) on 8 trn2 NeuronCores.

Sharding: data-parallel over batch (2) x tensor-parallel over heads (4 heads/core).
Per-core dense partials are summed with a chunked fp8 ReduceScatter over each
batch group ([[0,1,2,3],[4,5,6,7]]); each core finishes residual+LayerNorm on its
own token shard and the host reassembles the full [2, 2048, 1024] output.

All matmuls except the 64-deep QK^T scores run as fp8e4 DoubleRow (two
contraction elements per PE cell per cycle -> 2x throughput); scores stay bf16
since a 64-deep contraction cannot use the extra rows.  Weights are pre-scaled
x32 on the host so fp8 operands sit in e4m3's normal range, and the scale is
divided back out by the exp scale (scores), the denominator ratio (V path),
and the dense evacuation (x1/32) + scale-invariant LayerNorm (output path).
"""

import sys

for _p in ("/opt/trn_rl_repo",):
    if _p not in sys.path:
        sys.path.insert(0, _p)

import numpy as np
import ml_dtypes

import concourse.bass as bass
import concourse.mybir as mybir
import concourse.tile as tile
from concourse import bacc
from concourse.bass_utils import run_bass_kernel_spmd

BF16 = ml_dtypes.bfloat16
FP8 = ml_dtypes.float8_e4m3  # TRN float8e4: max normal 240, like IEEE e4m3

HIDDEN = 1024
HEADS = 16
HD = 64  # head dim
B = 2
S = 2048
LN_EPS = 1e-5

N_CORES = 8
TP = 4  # tensor-parallel ranks per batch group
LHEADS = HEADS // TP  # 4 local heads
PAIRS = LHEADS // 2  # 2 head pairs
SHARD = S // TP  # 512 tokens of final output per core
NCD = HIDDEN // 128  # 8 contraction chunks
NTOK = S // 128  # 16 key chunks
NT2 = NTOK // 2  # 8 DoubleRow key chunk-pairs
NQT = 4  # attention q-tiles (512 q each)
QT = S // NQT  # 512
REPLICA_GROUPS = [[0, 1, 2, 3], [4, 5, 6, 7]]
# ReduceScatter chunk boundaries in 128-token units.  4-rank collectives run
# the ring algorithm whose ~20us step floor dominates small payloads, so fewer
# bigger chunks beat many small ones; the tail is one 0.5MB fp8 RS.
RS_CHUNKS = [(0, 4), (4, 8), (8, 12), (12, 16)]
NCHUNK = len(RS_CHUNKS)
# per-rank rows per chunk (chunk token count / 4 ranks)
RS_SZ = [(hi - lo) * 32 for lo, hi in RS_CHUNKS]
# padded layout: chunk g's rows live at [g*128, g*128+sz) in hs_res / out
PAD_ROWS = NCHUNK * 128
# fp8 operand/wire scale: weights x32 host-side so fp8 values clear e4m3's
# subnormal cutoff; the dense ReduceScatter payload is 32x the true partial
# and scale-invariant LayerNorm (with eps x32^2) divides it back out
W_SCALE = 32.0
CC_SCALE = 32.0
# scores psum holds (32q)dot(32k); exp folds the /8 softmax scale and /1024
SCORE_SCALE = 0.125 / (W_SCALE * W_SCALE)
# Schraudolph exp-via-int-bits constants (y = bitcast_f32(round(x*A + B)) ~
# e^(x*SCORE_SCALE), rel err ~3%): offloads some exps from the saturated ACT
# engine to DVE (int mul-add) + GPSIMD (fp8 convert of the bitcast view)
SCH_A = float((1 << 23) * SCORE_SCALE / np.log(2.0))
SCH_B = float((1 << 23) * (127.0 - 0.0434609))

dt = mybir.dt
Alu = mybir.AluOpType
Act = mybir.ActivationFunctionType
DR = mybir.MatmulPerfMode.DoubleRow


def _build_program(debug_dumps=False):
    nc = bacc.Bacc(
        "TRN2", target_bir_lowering=False, debug=False, num_devices=N_CORES
    )

    # Route Exp and Ln to the one table set that holds both, so the kernel
    # never reloads ACT tables (set ids are positional; only values change).
    from concourse import hw_specs

    for name, funcs in hw_specs.get_activation_tables(nc.m.arch).items():
        if name != "natural_log_exp_and_others":
            funcs.discard(Act.Exp)
            funcs.discard(Act.Ln)

    # ---------------- DRAM I/O ----------------
    hsT = nc.dram_tensor("hsT", [HIDDEN, S], dt.float8e4, kind="ExternalInput")
    wqk = nc.dram_tensor("wqk", [HIDDEN, 512], dt.float8e4, kind="ExternalInput")
    wv = nc.dram_tensor("wv", [HIDDEN, 256], dt.float8e4, kind="ExternalInput")
    # w_dense pre-arranged for DoubleRow: [chan-in-pair 128, pair 2, hid 1024]
    wd = nc.dram_tensor("wd", [128, 2 * HIDDEN], dt.float8e4, kind="ExternalInput")
    bqk = nc.dram_tensor("bqk", [512, 1], dt.float32, kind="ExternalInput")
    hs_res = nc.dram_tensor(
        "hs_res", [PAD_ROWS, HIDDEN], dt.float32, kind="ExternalInput"
    )
    out = nc.dram_tensor("out", [PAD_ROWS, HIDDEN], dt.float32, kind="ExternalOutput")

    # internal DRAM for the collective (cannot use I/O tensors)
    cc_in = [
        nc.dram_tensor(f"cc_in{g}", [(hi - lo) * 128, HIDDEN], dt.float8e4)
        for g, (lo, hi) in enumerate(RS_CHUNKS)
    ]
    cc_out = [
        nc.dram_tensor(f"cc_out{g}", [RS_SZ[g], HIDDEN], dt.float8e4)
        for g in range(NCHUNK)
    ]

    with tile.TileContext(nc) as tc:
        with (
            tc.tile_pool(name="persist", bufs=1) as persist,
            tc.tile_pool(name="pT_pool", bufs=4) as pT_pool,
            tc.tile_pool(name="work", bufs=4) as work,
            tc.tile_pool(name="ln", bufs=2) as lnp,
            tc.tile_pool(name="psmm", bufs=2, space="PSUM") as psmm,
            tc.tile_pool(name="psq", bufs=2, space="PSUM") as psq,
            tc.tile_pool(name="psctx", bufs=1, space="PSUM") as psctx,
        ):
            # ---------------- persistent SBUF loads ----------------
            zero_sb = persist.tile([128, 1], dt.float32, name="zero_sb")
            nc.vector.memset(zero_sb, 0.0)
            nc.const_aps.aps[(dt.float32, 0.0)] = zero_sb
            eps_sb = persist.tile([128, 1], dt.float32, name="eps_sb")
            nc.vector.memset(eps_sb, LN_EPS * CC_SCALE * CC_SCALE)
            inv32_sb = persist.tile([128, 1], dt.float32, name="inv32_sb")
            nc.vector.memset(inv32_sb, 1.0 / 32.0)
            # input DMAs ordered by when compute needs them: the first QK
            # quarter needs all hidden chunks of q-tile 0's tokens plus its
            # wqk column block, so hsT is split by TOKENS and wqk goes first;
            # wd/res aren't needed until the dense/LN phases and go last
            bqk_all = persist.tile([128, 4], dt.float32, name="bqk_all")
            nc.sync.dma_start(
                out=bqk_all, in_=bqk[:, :].rearrange("(m p) o -> p (m o)", p=128)
            )
            wqk_all = persist.tile([128, NCD, 512], dt.float8e4, name="wqk_all")
            wqk_r = wqk[:, :].rearrange("(c p) n -> p c n", p=128)
            nc.sync.dma_start(out=wqk_all[:, :, 0:256], in_=wqk_r[:, :, 0:256])
            hsT_all = persist.tile([128, NCD, S], dt.float8e4, name="hsT_all")
            hsT_r = hsT[:, :].rearrange("(c p) t -> p c t", p=128)
            nc.sync.dma_start(out=hsT_all[:, :, 0:512], in_=hsT_r[:, :, 0:512])
            wv_all = persist.tile([128, NCD, 256], dt.float8e4, name="wv_all")
            nc.sync.dma_start(
                out=wv_all, in_=wv[:, :].rearrange("(c p) n -> p c n", p=128)
            )
            nc.sync.dma_start(out=hsT_all[:, :, 512:1024], in_=hsT_r[:, :, 512:1024])
            nc.sync.dma_start(out=hsT_all[:, :, 1024:2048], in_=hsT_r[:, :, 1024:2048])
            nc.sync.dma_start(out=wqk_all[:, :, 256:512], in_=wqk_r[:, :, 256:512])
            wd2_all = persist.tile([128, 2, HIDDEN], dt.float8e4, name="wd2_all")
            nc.sync.dma_start(
                out=wd2_all, in_=wd[:, :].rearrange("p (i n) -> p i n", i=2)
            )
            res_all = persist.tile([128, NCHUNK, HIDDEN], dt.float32, name="res_all")
            nc.sync.dma_start(
                out=res_all,
                in_=hs_res[:, :].rearrange("(g p) n -> p g n", p=128),
            )
            bqk_sb = [bqk_all[:, m : m + 1] for m in range(4)]
            # DoubleRow views: pair hidden chunk j with chunk j+4 so one pass
            # contracts 256 hidden dims (128 partitions x 2 per cell)
            hsT_dr = hsT_all.rearrange("p (i c) t -> p i c t", i=2)
            wqk_dr = wqk_all.rearrange("p (i c) n -> p i c n", i=2)
            wv_dr = wv_all.rearrange("p (i c) n -> p i c n", i=2)

            # qkT m-chunk layout: 0=K pair0, 1=Q pair0, 2=K pair1, 3=Q pair1
            # (partitions 0:64 = even head of the pair, 64:128 = odd head);
            # values are 32x-scaled (folded into wqk/bqk host-side)
            qkT_sb = [
                persist.tile([128, S], dt.bfloat16, name=f"qkT{m}") for m in range(4)
            ]
            # V tiles per key chunk-PAIR: [key-in-chunk 128, chunk-parity 2,
            # 4 heads x (32V(64) | ones(64))] in fp8 for DoubleRow ctx
            v2_sb = [
                persist.tile([128, 2, 512], dt.float8e4, name=f"v{t}")
                for t in range(NT2)
            ]
            # ctx^T fp8, 32x-scaled: [chan-in-pair 128, pair 2, tok S]
            ctxT2 = persist.tile([128, 2, S], dt.float8e4, name="ctxT2")

            # ---------------- projection emitters ----------------
            def emit_qk_quarter(m, q):
                ps = psq.tile([128, 512], dt.float32, name="ps_qk")
                for j in range(4):
                    nc.tensor.matmul(
                        ps,
                        lhsT=wqk_dr[:, :, j, m * 128 : (m + 1) * 128],
                        rhs=hsT_dr[:, :, j, q * 512 : (q + 1) * 512],
                        start=(j == 0),
                        stop=(j == 3),
                        perf_mode=DR,
                    )
                nc.vector.tensor_scalar_add(
                    out=qkT_sb[m][:, q * 512 : (q + 1) * 512],
                    in0=ps,
                    scalar1=bqk_sb[m],
                )

            def emit_v2_chunk(t2):
                for i in range(2):
                    t = 2 * t2 + i
                    ps = psq.tile([128, 512], dt.float32, name="ps_qk")
                    for j in range(4):
                        nc.tensor.matmul(
                            ps[:, 0:256],
                            lhsT=hsT_dr[:, :, j, t * 128 : (t + 1) * 128],
                            rhs=wv_dr[:, :, j, :],
                            start=(j == 0),
                            stop=(j == 3),
                            perf_mode=DR,
                        )
                    vt = v2_sb[t2][:, i, :].rearrange("p (g c) -> p g c", c=128)
                    nc.vector.tensor_copy(
                        out=vt[:, :, 0:64],
                        in_=ps[:, 0:256].rearrange("p (g c) -> p g c", c=64),
                    )
                vts = v2_sb[t2].rearrange("p i (g c) -> p i g c", c=128)
                nc.vector.memset(vts[:, :, :, 64:128], 1.0)

            # PE warmup: the HAM clock gate keeps an idle PE at half clock
            # and takes ~3.4us of sustained activity to release; burn dummy
            # matmuls during the input-DMA wait so the real stream runs warm
            dummy_sb = persist.tile([128, 512], dt.bfloat16, name="dummy_sb")
            nc.vector.memset(dummy_sb, 0.0)
            for _ in range(12):
                ps_w = psq.tile([128, 512], dt.float32, name="ps_qk")
                nc.tensor.matmul(
                    ps_w[0:1, :], lhsT=zero_sb[:, :].bitcast(dt.bfloat16)[:, 0:1],
                    rhs=dummy_sb, start=True, stop=True,
                )
            # Minimum prefix before attention can start: K pair0 q-tile 0
            # (covers scores kc 0..3) and Q pair0 q-tile 0; everything else
            # is woven into the attention loops' PE slack so exp starts as
            # soon as the first token quarter lands.
            emit_qk_quarter(0, 0)
            emit_qk_quarter(1, 0)

            # ---------------- phase 2: attention + dense + RS ----------------
            cc_insts = []
            dense_state = {"last_evac": None}

            def emit_dense_ti(ti_g):
                tok = ti_g * 128
                dsb = work.tile([128, 1024], dt.float8e4, name="dsb")
                for j in range(2):
                    ps_d = psq.tile([128, 512], dt.float32, name="ps_qk")
                    nc.tensor.matmul(
                        ps_d,
                        lhsT=ctxT2[:, :, tok : tok + 128],
                        rhs=wd2_all[:, :, j * 512 : (j + 1) * 512],
                        start=True,
                        stop=True,
                        perf_mode=DR,
                    )
                    # psum = (32 ctx)(32 wd) = 1024x partial; wire wants 32x
                    dense_state["last_evac"] = nc.vector.tensor_scalar_mul(
                        out=dsb[:, j * 512 : (j + 1) * 512],
                        in0=ps_d,
                        scalar1=1.0 / 32.0,
                    )
                g = next(
                    i for i, (lo, hi) in enumerate(RS_CHUNKS) if lo <= ti_g < hi
                )
                lo = RS_CHUNKS[g][0]
                nc.sync.dma_start(
                    out=cc_in[g][(ti_g - lo) * 128 : (ti_g - lo + 1) * 128, :],
                    in_=dsb,
                )
                if ti_g == RS_CHUNKS[g][1] - 1:
                    cc_insts.append(
                        nc.gpsimd.collective_compute(
                            "ReduceScatter",
                            Alu.add,
                            replica_groups=REPLICA_GROUPS,
                            ins=[cc_in[g][:, :].opt()],
                            outs=[cc_out[g][:, :].opt()],
                        )
                    )

            # filler schedule: (qt, pair, kc) -> callables emitting ~0.9us of
            # PE work each, consumed after that kc-pair's ctx matmuls.
            # V chunk-pairs must precede their use in qt0-pair0's ctx; qk
            # quarters must precede the (qt, pair) that reads them; dense
            # ti's trail their q-tile by one qt.
            fill = {}

            def _add(qt, p, kc, fn):
                fill.setdefault((qt, p, kc), []).append(fn)

            for t2 in range(6):  # V chunk-pairs 2..7 during qt0-pair0
                _add(0, 0, 2 * t2, (lambda t=t2 + 2: emit_v2_chunk(t)))
            for kc, (m, q) in [
                (0, (0, 1)), (1, (2, 1)), (2, (0, 2)), (3, (0, 3)),
                (4, (2, 0)), (6, (3, 0)),
            ]:
                _add(0, 0, kc, (lambda m=m, q=q: emit_qk_quarter(m, q)))
            for kc, (m, q) in [
                (0, (2, 2)), (1, (1, 1)), (2, (3, 1)), (4, (2, 3)),
            ]:
                _add(0, 1, kc, (lambda m=m, q=q: emit_qk_quarter(m, q)))
            _add(1, 0, 5, lambda: emit_qk_quarter(1, 2))
            _add(1, 1, 4, lambda: emit_qk_quarter(3, 2))
            _add(2, 0, 5, lambda: emit_qk_quarter(1, 3))
            _add(2, 1, 4, lambda: emit_qk_quarter(3, 3))
            # dense for q-tile qt woven into qt+1 pair0 (kc>=4 so the
            # previous tile's ctxT normalize on DVE has drained first)
            for qt in range(1, NQT):
                for i in range(4):
                    _add(qt, 0, 4 + 2 * i, (lambda ti=(qt - 1) * 4 + i: emit_dense_ti(ti)))

            # Whole kc-pairs offloaded from the saturated ACT engine to DVE:
            # exp is computed as Schraudolph int-bits on DVE and the ctx
            # matmul reads the int32 buffer through a truncated-bf16 view
            # (the high half of each fp32), so no convert op is needed.
            # t2=3 of every (qt, pair) except qt0-pair0 (its DVE is already
            # loaded with the woven V2 copies).
            OFF_T2 = 3
            offload = set()  # measured: DVE can't absorb the work in-window
            ibs = {}

            def emit_exp(pT_slice, ps, qt, p, kc):
                if (qt, p) in offload and kc // 2 == OFF_T2:
                    ib = work.tile([128, 1024], dt.int32, name="schb")
                    nc.vector.tensor_scalar(
                        out=ib, in0=ps, scalar1=SCH_A, scalar2=SCH_B,
                        op0=Alu.mult, op1=Alu.add,
                    )
                    ibs[(qt, p, kc)] = ib
                else:
                    nc.scalar.activation(
                        out=pT_slice, in_=ps, func=Act.Exp, scale=SCORE_SCALE
                    )

            for qt in range(NQT):
                for p in range(PAIRS):
                    km = 2 * p  # K m-chunk
                    qm = 2 * p + 1  # Q m-chunk
                    ctx_ps = [
                        psctx.tile([128, 512], dt.float32, name=f"ps_ctx{l}")
                        for l in range(2)
                    ]

                    def emit_scores(kc, km=km, qm=qm, qt=qt):
                        ps_s = psmm.tile([128, 1024], dt.float32, name="ps_mm")
                        # scores^T for both heads of the pair (concurrent row
                        # groups: even head rows 0:64, odd head rows 64:128)
                        for l in range(2):
                            nc.tensor.matmul(
                                ps_s[:, l * 512 : (l + 1) * 512],
                                lhsT=qkT_sb[km][
                                    l * 64 : (l + 1) * 64, kc * 128 : (kc + 1) * 128
                                ],
                                rhs=qkT_sb[qm][
                                    l * 64 : (l + 1) * 64, qt * 512 : (qt + 1) * 512
                                ],
                                start=True,
                                stop=True,
                                tile_position=(l * 64, 0),
                            )
                        return ps_s

                    # software pipeline: scores run one k-chunk ahead so the
                    # PE never sits in-order behind ctx's wait on exp
                    ps_a = emit_scores(0)
                    ps_b = emit_scores(1)
                    if qt == 0 and p == 0:
                        emit_v2_chunk(0)
                        emit_v2_chunk(1)
                    for t2 in range(NT2):
                        pT2 = pT_pool.tile([128, 2, 1024], dt.float8e4, name="pT")
                        emit_exp(pT2[:, 0, :], ps_a, qt, p, 2 * t2)
                        ps_a = (
                            emit_scores(2 * t2 + 2) if 2 * t2 + 2 < NTOK else None
                        )
                        emit_exp(pT2[:, 1, :], ps_b, qt, p, 2 * t2 + 1)
                        ps_b = (
                            emit_scores(2 * t2 + 3) if 2 * t2 + 3 < NTOK else None
                        )
                        # ctx^T (+ denominator rows 64:128) over the key
                        # chunk-pair: DoubleRow contracts 256 keys per pass
                        if (qt, p) in offload and t2 == OFF_T2:
                            for i in range(2):
                                ib = ibs.pop((qt, p, 2 * t2 + i))
                                pbf = ib[:, :].bitcast(dt.bfloat16).rearrange(
                                    "q (f two) -> q f two", two=2
                                )[:, :, 1]
                                for l in range(2):
                                    h = 2 * p + l
                                    nc.tensor.matmul(
                                        ctx_ps[l],
                                        lhsT=v2_sb[t2][:, i, h * 128 : (h + 1) * 128],
                                        rhs=pbf[:, l * 512 : (l + 1) * 512],
                                        start=False,
                                        stop=False,
                                    )
                        else:
                            for l in range(2):
                                h = 2 * p + l
                                nc.tensor.matmul(
                                    ctx_ps[l],
                                    lhsT=v2_sb[t2][:, :, h * 128 : (h + 1) * 128],
                                    rhs=pT2[:, :, l * 512 : (l + 1) * 512],
                                    start=(t2 == 0),
                                    stop=(t2 == NT2 - 1),
                                    perf_mode=DR,
                                )
                        for fn in fill.get((qt, p, 2 * t2), ()):
                            fn()
                        for fn in fill.get((qt, p, 2 * t2 + 1), ()):
                            fn()
                    # normalize: 32V num [0:64] / den [64:128] -> ctxT2 (fp8)
                    for l in range(2):
                        den_sb = work.tile([64, 512], dt.float32, name="den_sb")
                        nc.vector.tensor_copy(
                            out=den_sb, in_=ctx_ps[l][64:128, :]
                        )
                        rec = work.tile([64, 512], dt.float32, name="rec")
                        nc.vector.reciprocal_approx_fast(out=rec, in_=den_sb)
                        nc.vector.tensor_tensor(
                            out=ctxT2[
                                l * 64 : (l + 1) * 64, p, qt * 512 : (qt + 1) * 512
                            ],
                            in0=ctx_ps[l][0:64, :],
                            in1=rec,
                            op=Alu.mult,
                        )
            # last q-tile's dense has no following attention to hide in
            for ti in range(4):
                emit_dense_ti(12 + ti)
            last_evac = dense_state["last_evac"]

            # ---------------- phase 3: residual + LayerNorm ----------------
            # Pin every LN chunk after the last dense evacuation so the
            # in-order engine queues never block on an RS mid-attention;
            # LN for the earlier chunks then fills the final RS wait.
            from concourse.bass import _add_dep_helper

            for g in range(NCHUNK):
                sz = RS_SZ[g]
                xb = lnp.tile([128, HIDDEN], dt.float8e4, name="xb")
                xb_dma = nc.sync.dma_start(out=xb[:sz, :], in_=cc_out[g][:, :])
                _add_dep_helper(
                    xb_dma.ins,
                    last_evac.ins,
                    sync=True,
                    reason="LN after attention/dense (keep queues unblocked)",
                )
                x = lnp.tile([128, HIDDEN], dt.float32, name="x")
                nc.vector.tensor_tensor(
                    out=x[:sz, :],
                    in0=xb[:sz, :],
                    in1=res_all[:sz, g, :],
                    op=Alu.add,
                )
                stats = lnp.tile([128, 2, 6], dt.float32, name="stats")
                xv = x.rearrange("p (s f) -> p s f", f=512)
                for i in range(2):
                    nc.vector.bn_stats(out=stats[:sz, i, :], in_=xv[:sz, i, :])
                mv = lnp.tile([128, 2], dt.float32, name="mv")
                nc.vector.bn_aggr(out=mv[:sz, :], in_=stats[:sz, :, :])
                # rstd = exp(-0.5 * ln(var + eps)) -- stays in the exp/ln table set
                lnv = lnp.tile([128, 1], dt.float32, name="lnv")
                nc.scalar.activation(
                    out=lnv[:sz, :], in_=mv[:sz, 1:2], func=Act.Ln, bias=eps_sb[:sz, :]
                )
                rstd = lnp.tile([128, 1], dt.float32, name="rstd")
                nc.scalar.activation(
                    out=rstd[:sz, :], in_=lnv[:sz, :], func=Act.Exp, scale=-0.5
                )
                y = lnp.tile([128, HIDDEN], dt.float32, name="y")
                nc.vector.tensor_scalar(
                    out=y[:sz, :],
                    in0=x[:sz, :],
                    scalar1=mv[:sz, 0:1],
                    scalar2=rstd[:sz, :],
                    op0=Alu.subtract,
                    op1=Alu.mult,
                )
                nc.sync.dma_start(
                    out=out[g * 128 : g * 128 + sz, :], in_=y[:sz, :]
                )

    nc.compile()
    return nc


_PROGRAM = None


def _get_program():
    global _PROGRAM
    if _PROGRAM is None:
        _PROGRAM = _build_program()
    return _PROGRAM


def _prep_core_inputs(hidden_states, w_qkv, b_qkv, w_dense, b_dense):
    """Build the 8 per-core input maps (numpy, host-side sharding)."""
    hs = np.asarray(hidden_states, dtype=np.float32)
    w_qkv = np.asarray(w_qkv, dtype=np.float32)
    b_qkv = np.asarray(b_qkv, dtype=np.float32)
    w_dense = np.asarray(w_dense, dtype=np.float32)
    b_dense = np.asarray(b_dense, dtype=np.float32)

    # v-channel bias folded into a host-side output bias:
    # b_out = b_dense + b_v_full @ w_dense   (b_v in ctx channel order)
    bv_full = np.empty((HIDDEN,), dtype=np.float64)
    for g in range(HEADS):
        bv_full[g * HD : (g + 1) * HD] = b_qkv[g * 192 + 128 : g * 192 + 192]
    # w_dense rows are already in (head, d) = g*64+d order, matching bv_full
    b_out = (
        b_dense.astype(np.float64)
        + bv_full @ w_dense.astype(np.float64)
    ).astype(np.float32)

    in_maps = []
    for r in range(N_CORES):
        b = r // TP
        tp = r % TP
        gheads = [4 * tp + l for l in range(LHEADS)]

        hsT_f8 = np.ascontiguousarray(hs[b].T).astype(FP8)  # [1024, 2048]

        # wqk column order: per pair: K(even) K(odd) Q(even) Q(odd), 64 each
        wqk_cols = np.empty((HIDDEN, 512), dtype=np.float32)
        bqk_vec = np.empty((512,), dtype=np.float32)
        for p in range(PAIRS):
            for l in range(2):
                g = gheads[2 * p + l]
                kcol = slice(g * 192 + 64, g * 192 + 128)
                qcol = slice(g * 192, g * 192 + 64)
                base = p * 256
                wqk_cols[:, base + l * 64 : base + (l + 1) * 64] = w_qkv[:, kcol]
                wqk_cols[:, base + 128 + l * 64 : base + 128 + (l + 1) * 64] = w_qkv[
                    :, qcol
                ]
                bqk_vec[base + l * 64 : base + (l + 1) * 64] = b_qkv[kcol]
                bqk_vec[base + 128 + l * 64 : base + 128 + (l + 1) * 64] = b_qkv[qcol]

        wv_cols = np.empty((HIDDEN, 256), dtype=np.float32)
        for l, g in enumerate(gheads):
            wv_cols[:, l * 64 : (l + 1) * 64] = w_qkv[
                :, g * 192 + 128 : g * 192 + 192
            ]

        # head-ordered dense rows, DoubleRow layout [chan-in-pair, pair, hid]
        wd_rows = np.empty((256, HIDDEN), dtype=np.float32)
        for l, g in enumerate(gheads):
            wd_rows[l * 64 : (l + 1) * 64, :] = w_dense[g * 64 : (g + 1) * 64, :]
        wd2 = np.ascontiguousarray(
            (wd_rows * W_SCALE).reshape(2, 128, HIDDEN).transpose(1, 0, 2)
        ).reshape(128, 2 * HIDDEN)

        # residual shard (+ folded output bias); padded layout: chunk g's
        # sz rows live at [g*128, g*128+sz), covering global tokens
        # lo*128 + tp*sz + [0, sz)
        res = np.zeros((PAD_ROWS, HIDDEN), dtype=np.float32)
        for g, (lo, hi) in enumerate(RS_CHUNKS):
            sz = RS_SZ[g]
            t0 = lo * 128 + tp * sz
            res[g * 128 : g * 128 + sz, :] = (
                hs[b, t0 : t0 + sz, :] + b_out
            ) * CC_SCALE

        in_maps.append(
            {
                "hsT": hsT_f8,
                "wqk": (wqk_cols * W_SCALE).astype(FP8),
                "wv": (wv_cols * W_SCALE).astype(FP8),
                "wd": wd2.astype(FP8),
                "bqk": (bqk_vec * W_SCALE).reshape(512, 1),
                "hs_res": res,
            }
        )
    return in_maps


def kernel(hidden_states, w_qkv, b_qkv, w_dense, b_dense, ln_gamma, ln_beta,
           _return_perf=False, **run_kwargs):
    ln_gamma = np.asarray(ln_gamma, dtype=np.float32)
    ln_beta = np.asarray(ln_beta, dtype=np.float32)
    gamma_one = np.allclose(ln_gamma, 1.0)
    beta_zero = np.allclose(ln_beta, 0.0)

    nc = _get_program()
    in_maps = _prep_core_inputs(hidden_states, w_qkv, b_qkv, w_dense, b_dense)
    res = run_bass_kernel_spmd(
        nc, in_maps, core_ids=list(range(N_CORES)), **run_kwargs
    )

    full = np.empty((B, S, HIDDEN), dtype=np.float32)
    for r in range(N_CORES):
        b = r // TP
        tp = r % TP
        o = res.results[r]["out"]
        for g, (lo, hi) in enumerate(RS_CHUNKS):
            sz = RS_SZ[g]
            t0 = lo * 128 + tp * sz
            full[b, t0 : t0 + sz, :] = o[g * 128 : g * 128 + sz, :]

    if not (gamma_one and beta_zero):
        # spec fills gamma=ones, beta=zeros; fall back on host if they differ
        full = full * ln_gamma[None, None, :] + ln_beta[None, None, :]

    if _return_perf:
        return full, res
    return full
